# revision 21
# baseline (speedup 1.0000x reference)
"""Trainium2 Bass kernel for the chain-DAG generator MLP.

Math (per batch row b, node i in topological order 0..15):
    c_i = input_c @ Wc[:, 16i:16i+16]
    d_i = input_d @ theta[:, 16i:16i+16],  theta = mu + softplus(sigma)*noise_d
    h_i = relu(c_i @ W1c_i + d_i @ W1d_i + n_i @ W1n_i + p_i * w_p_i + b1_i)
    out_i = h_i @ W2_i + b2_i,   p_i = out_{i-1} for i in 1..13 (0,14,15 roots)

Device mapping (data-parallel over batch on 8 cores, B_s=16384 rows/core,
32 chunks of 512 batch columns, hidden-on-partition layout, all-bf16
operands with fp32 PSUM accumulation):

  - PSUM-write-port-aware tiling: the PE drains at most 128 results/cycle,
    so full-M=128 matmuls never overlap. Bases stay M=128 (both pair
    nodes real). Chain matmuls are M=64 (child half only) so the two
    in-flight chunks' chains (opposite column halves, STAG=10 with the
    period-4 strip pattern) can run concurrently. Collects are M=32,
    4-way column-tiled into ONE PSUM bank (node i -> partition
    32*(i%4) + i//4), giving ~4x collect concurrency.
  - base per pair q: ONE K=48 bf16 matmul (Wc/theta/b1 folded into lhsT,
    bias via ones row) into PSUM bank [128, 512] at row strip 64*(q%2).
  - chain: relu(h_i) (Act/DVE round-robin) into SBUF at partitions
    64*s_i, s_i = ((i+1)//2)%2; child_pre += outer(W2_i, w_p_{i+1})^T @
    h_i; parent b2 folded into child bias on host.
  - collect: at chunk end, 16 M=32 matmuls into one bank (single
    start=True on the first; has_written is per-element so later column
    groups overwrite-then-accumulate correctly); one Act/DVE evacuation
    [128,512] and 4 strided DMAs; b2 + row permutation undone on host.
  - 2-chunk software pipelining (STAG=10): chains of the two chunks are
    emitted adjacently each step so the PE stream stays dense.
"""

import threading

import ml_dtypes
import numpy as np

import concourse.bacc as bacc
import concourse.mybir as mybir
from concourse.bass_utils import run_bass_kernel_spmd
from concourse.tile import TileContext

N_CORES = 8
B_FULL = 131072
B_S = B_FULL // N_CORES  # 16384
CHUNK = 512
I_DIM = 16
N_PAIRS = 8
STAG = 10  # node-step stagger between in-flight chunks (2 mod 4!)

F32 = mybir.dt.float32
BF = mybir.dt.bfloat16
BF16 = ml_dtypes.bfloat16

# row strip (0/1) of node i's hidden inside its pair bank; also the SBUF
# strip of h_i and the PE row group of every matmul reading h_i.
S_STRIP = [((i + 1) // 2) % 2 for i in range(I_DIM)]
# PE row group of all matmuls accumulating into pair bank q
P_STRIP = [q % 2 for q in range(N_PAIRS)]
COLLECT_A = [i for i in range(I_DIM) if S_STRIP[i] == 0]
COLLECT_B = [i for i in range(I_DIM) if S_STRIP[i] == 1]
# device OUT rows 0:8 hold COLLECT_A nodes, 8:16 hold COLLECT_B
OUT_PERM = COLLECT_A + COLLECT_B


def build_nc(b_s: int = B_S, chunk: int = CHUNK):
    """Build the single-core program (SPMD: same program on all cores)."""
    assert b_s % chunk == 0
    n_chunks = b_s // chunk

    nc = bacc.Bacc(
        "TRN2", target_bir_lowering=False, debug=False, num_devices=N_CORES
    )

    # Per-core inputs
    s_d = nc.dram_tensor("S", [16, b_s], BF, kind="ExternalInput").ap()
    nt_d = nc.dram_tensor("NT", [256, b_s], BF, kind="ExternalInput").ap()
    # Folded weights (replicated on every core)
    px_d = nc.dram_tensor("PX", [128, 128 * N_PAIRS], BF, kind="ExternalInput").ap()
    mc_d = nc.dram_tensor("MC", [128, 64 * 13], BF, kind="ExternalInput").ap()
    cl_d = nc.dram_tensor("CLW", [128, 32 * 16], BF, kind="ExternalInput").ap()
    out_d = nc.dram_tensor("OUT", [16, b_s], F32, kind="ExternalOutput").ap()
    COLLECT_AB = (COLLECT_A, COLLECT_B)

    with TileContext(nc) as tc:
        with (
            tc.tile_pool(name="consts", bufs=1) as cpool,
            tc.tile_pool(name="ins", bufs=18) as ipool,
            tc.tile_pool(name="hbuf", bufs=28) as hpool,
            tc.tile_pool(name="obuf", bufs=2) as opool,
            tc.tile_pool(name="pairs", bufs=6, space="PSUM") as ppool,
            tc.tile_pool(name="outp", bufs=1, space="PSUM") as qpool,
        ):
            px_t = cpool.tile([128, 128 * N_PAIRS], BF)
            nc.sync.dma_start(out=px_t[:, :], in_=px_d[:, :])
            mc_t = cpool.tile([128, 64 * 13], BF)
            nc.sync.dma_start(out=mc_t[:, :], in_=mc_d[:, :])
            cl_t = cpool.tile([128, 32 * 16], BF)
            nc.sync.dma_start(out=cl_t[:, :], in_=cl_d[:, :])

            banks = {}  # (ch, q) -> pair bank tile
            hs = {}  # (ch, i) -> h tile
            relu_rr = [0]  # global round-robin over Act/DVE

            xs = {}  # (ch, q) -> x tile (prefetched)

            def emit_xdma(ch, q):
                c0 = ch * chunk
                sl = slice(c0, c0 + chunk)
                R = 64 * P_STRIP[q]
                x_q = ipool.tile([128, chunk], BF, tag="x", name=f"x_{ch}_{q}")
                nc.sync.dma_start(out=x_q[R : R + 16, :], in_=s_d[:, sl])
                nc.sync.dma_start(
                    out=x_q[R + 16 : R + 48, :], in_=nt_d[32 * q : 32 * q + 32, sl]
                )
                xs[(ch, q)] = x_q

            def emit_base(ch, q):
                R = 64 * P_STRIP[q]
                x_q = xs[(ch, q)]
                bank = ppool.tile(
                    [128, chunk], F32, tag="bank", name=f"bank_{ch}_{q}"
                )
                banks[(ch, q)] = bank
                nc.tensor.matmul(
                    out=bank[:, :],
                    lhsT=px_t[R : R + 48, 128 * q : 128 * (q + 1)],
                    rhs=x_q[R : R + 48, :],
                    start=True,
                    stop=(q == 7),  # bank 7 takes no chain matmul
                    skip_group_check=True,
                )

            def emit_pre(ch, i):
                """Prefetched-x base issue + relu for node i of chunk ch."""
                q, s = i // 2, S_STRIP[i]
                if i == 0:
                    emit_xdma(ch, 0)
                    emit_xdma(ch, 1)
                    emit_base(ch, 0)
                if i % 2 == 1:
                    p = (i + 1) // 2
                    if p <= 7:
                        emit_base(ch, p)
                    if p + 1 <= 7:
                        emit_xdma(ch, p + 1)
                h = hpool.tile([128, chunk], BF, tag="h", name=f"h_{ch}_{i}")
                hs[(ch, i)] = h
                rows = slice(64 * s, 64 * s + 64)
                if relu_rr[0] % 2 == 0:
                    nc.scalar.activation(
                        h[rows, :],
                        banks[(ch, q)][rows, :],
                        mybir.ActivationFunctionType.Relu,
                    )
                else:
                    nc.vector.tensor_scalar_max(
                        out=h[rows, :], in0=banks[(ch, q)][rows, :], scalar1=0.0
                    )
                relu_rr[0] += 1

            def emit_chain(ch, i):
                if i > 12:
                    return
                s, sc = S_STRIP[i], S_STRIP[i + 1]
                rows = slice(64 * s, 64 * s + 64)
                crows = slice(64 * sc, 64 * sc + 64)
                nc.tensor.matmul(
                    out=banks[(ch, (i + 1) // 2)][crows, :],
                    lhsT=mc_t[rows, 64 * i : 64 * (i + 1)],
                    rhs=hs[(ch, i)][rows, :],
                    start=False,
                    stop=(i % 2 == 0),  # chain(2q) closes pair q's group
                    skip_group_check=True,
                )

            def emit_collect(ch):
                c0 = ch * chunk
                sl = slice(c0, c0 + chunk)
                # two banks (A: strip-0 nodes, B: strip-1); M=32 tiles at
                # col position 0 — adjacent A/B matmuls are on different
                # row groups AND different banks, so they overlap safely
                bo = {
                    0: qpool.tile([32, chunk], F32, tag="boA", name=f"boA_{ch}"),
                    1: qpool.tile([32, chunk], F32, tag="boB", name=f"boB_{ch}"),
                }
                for k in range(8):
                    for g in (0, 1):
                        i = COLLECT_AB[g][k]
                        rows = slice(64 * S_STRIP[i], 64 * S_STRIP[i] + 64)
                        nc.tensor.matmul(
                            out=bo[g][0:32, :],
                            lhsT=cl_t[rows, 32 * i : 32 * (i + 1)],
                            rhs=hs[(ch, i)][rows, :],
                            start=(k == 0),
                            stop=(k == 7),
                            skip_group_check=True,
                        )
                o_ta = opool.tile([8, chunk], F32, tag="oa")
                o_tb = opool.tile([8, chunk], F32, tag="ob")
                nc.scalar.copy(out=o_ta[:, :], in_=bo[0][0:8, :])
                nc.vector.tensor_copy(out=o_tb[:, :], in_=bo[1][0:8, :])
                nc.sync.dma_start(out=out_d[0:8, sl], in_=o_ta[:, :])
                nc.sync.dma_start(out=out_d[8:16, sl], in_=o_tb[:, :])
                for kk in [k for k in banks if k[0] == ch]:
                    del banks[kk]
                for kk in [k for k in hs if k[0] == ch]:
                    del hs[kk]
                for kk in [k for k in xs if k[0] == ch]:
                    del xs[kk]

            # chains/collects are emitted one step AFTER the relu they
            # depend on, so the PE never waits on a just-issued Act/DVE
            # op (keeps the matmul stream dense and the clock warm)
            n_steps = I_DIM + STAG * (n_chunks - 1) + 1
            prev = []
            for t in range(n_steps):
                act = [
                    (ch, t - STAG * ch)
                    for ch in range(n_chunks)
                    if 0 <= t - STAG * ch < I_DIM
                ]
                for ch, i in prev:
                    emit_chain(ch, i)
                for ch, i in prev:
                    if i == 15:
                        emit_collect(ch)
                for ch, i in act:
                    emit_pre(ch, i)
                prev = act

    nc.compile()
    return nc


def prep_weights(noise_d, mu, sigma, Wc, W1, b1, W2, b2):
    """Fold the tiny parameter tensors into the device weight layout."""
    theta = mu + np.log1p(np.exp(sigma)) * noise_d  # [4, 256]
    w_p = W1[:, 48, :]  # [16, 64]
    b1e = b1.copy()  # [16, 64]
    for i in range(1, 14):  # nodes with parent i-1
        b1e[i] = b1[i] + w_p[i] * b2[i - 1]

    # base lhsT per pair at rows 64*P_STRIP[q]:
    # [A_c(10); A_d(4); b1e(1); 0(1); A_n block-diag(32)]; node i's 64
    # M-columns at 64*S_STRIP[i].
    px = np.zeros((128, 128 * N_PAIRS), np.float32)
    for q in range(N_PAIRS):
        R = 64 * P_STRIP[q]
        for r in range(2):
            i = 2 * q + r
            cols = slice(128 * q + 64 * S_STRIP[i], 128 * q + 64 * S_STRIP[i] + 64)
            px[R + 0 : R + 10, cols] = Wc[:, 16 * i : 16 * (i + 1)] @ W1[i, 0:16, :]
            px[R + 10 : R + 14, cols] = (
                theta[:, 16 * i : 16 * (i + 1)] @ W1[i, 16:32, :]
            )
            px[R + 14, cols] = b1e[i]
            px[R + 16 + 16 * r : R + 32 + 16 * r, cols] = W1[i, 32:48, :]

    # chain lhsT for node i -> child i+1 at rows 64*S_STRIP[i]; M=64
    # (child half only)
    mc = np.zeros((128, 64 * 13), np.float32)
    for i in range(13):
        R = 64 * S_STRIP[i]
        mc[R : R + 64, 64 * i : 64 * (i + 1)] = np.outer(W2[i], w_p[i + 1])

    # collect lhsT: [64, 32] per node at rows 64*S_STRIP[i]; real column
    # is the node's index within its strip group (out partitions 0:8)
    cl = np.zeros((128, 32 * 16), np.float32)
    for i in range(16):
        R = 64 * S_STRIP[i]
        grp = COLLECT_A if S_STRIP[i] == 0 else COLLECT_B
        cl[R : R + 64, 32 * i + grp.index(i)] = W2[i]

    return {
        "PX": px.astype(BF16),
        "MC": mc.astype(BF16),
        "CLW": cl.astype(BF16),
    }


def prep_core_inputs(noise, input_c, input_d, c):
    """Shard + transpose per-core batch inputs."""
    b0, b1_ = c * B_S, (c + 1) * B_S
    s = np.zeros((16, B_S), np.float32)
    s[0:10] = input_c[b0:b1_].T
    s[10:14] = input_d[b0:b1_].T
    s[14] = 1.0
    nt = np.ascontiguousarray(noise[b0:b1_].T)
    return {"S": s.astype(BF16), "NT": nt.astype(BF16)}


_NC_LOCK = threading.Lock()
_NC_CACHE = {}


def _get_nc():
    with _NC_LOCK:
        if "nc" not in _NC_CACHE:
            _NC_CACHE["nc"] = build_nc()
        return _NC_CACHE["nc"]


def kernel(noise, input_c, input_d, noise_d, mu, sigma, Wc, W1, b1, W2, b2):
    noise = np.asarray(noise, np.float32)
    input_c = np.asarray(input_c, np.float32)
    input_d = np.asarray(input_d, np.float32)
    b2 = np.asarray(b2, np.float32)
    w = prep_weights(
        np.asarray(noise_d, np.float32),
        np.asarray(mu, np.float32),
        np.asarray(sigma, np.float32),
        np.asarray(Wc, np.float32),
        np.asarray(W1, np.float32),
        np.asarray(b1, np.float32),
        np.asarray(W2, np.float32),
        b2,
    )
    in_maps = []
    for c in range(N_CORES):
        m = prep_core_inputs(noise, input_c, input_d, c)
        m.update(w)
        in_maps.append(m)

    nc = _get_nc()
    res = run_bass_kernel_spmd(nc, in_maps, list(range(N_CORES)))
    out_p = np.concatenate(
        [res.results[c]["OUT"].T for c in range(N_CORES)], axis=0
    )
    # undo the device row permutation; b2 is added on host (device
    # computes s_i = h_i @ W2_i only)
    out = np.empty_like(out_p)
    out[:, OUT_PERM] = out_p
    out = out + b2[None, :]
    return np.ascontiguousarray(out, np.float32)
